# revision 33
# baseline (speedup 1.0000x reference)
"""MixLoss Trainium2 kernel.

loss = 0.5*(ce + nll) over tokens, with
  ce  = -mean[ log_softmax_c(segment_max_f(logits))[label] ]
  nll = -mean[ log((softmax_f(logits) @ mask)[label]) ]

Data-parallel over 8 cores (batch split). Per core: 8192 tokens = 64 tiles
of 128 (tokens on SBUF partitions).

The host permutes the fine-class axis into a 2-level FOLD-STABLE layout:
each coarse class gets a capacity cap_c = 4*ceil(size_c/4); class c owns
residues [off_c, off_c + cap_c/4) of R = W/4, and its members occupy slots
{r + k*R : k in 0..4} for its residues r (pads hold logit -1000 -> exp 0,
neutral for both the group max and the group sum since all real E > 0).
Within a chunk of CH tiles the SBUF layout is w-major/tile-minor
([p, w*CH + t]) so every fold is a single fully-packed bf16 TensorTensor
(DVE 2x mode: 0.52 ns/elem in the cost model vs 1.04 for TensorReduce):

  fold1:  [0:W/2*CH] (op) [W/2*CH:W*CH]      (pairs w, w+2R)
  fold2:  [0:W/4*CH] (op) [W/4*CH:W/2*CH]    (pairs w, w+R)
  ragged tier reduce over g = cap/4 residues: pairwise TT trees on the
  Pool engine (otherwise idle) -> em/s slices of [p, ch*C*CH + c*CH + t]

Tail epilogue (batched once, off the pipeline's critical path):
  sum_em = sum_c EM, Z = sum_c S (Z = full row-sum of E since the segments
  partition the row), G = EM*S*onehot, num = sum_c G = EM[lab]*S[lab],
  den = sum_em*Z, term = ln(num) - ln(den) = logp_max[lab]+logp_coarse[lab].
Host sums partials: loss = -0.5 * sum(term) / n_tok.

bf16 logits / E / EM / S storage gives ~0.4% zero-mean rounding noise per
token which averages out over 65536 tokens (baseline did the same).
"""

import ml_dtypes
import numpy as np

import concourse.bacc as bacc
import concourse.mybir as mybir
from concourse import tile
from concourse.bass_utils import run_bass_kernel_spmd

N_CORES = 8
P = 128  # SBUF partitions = tokens per tile
CH = 8   # tiles per chunk (steady state)
TB = 16  # tiles per tree/epilogue block
GRAD_SIZES = [2, 2, 4, 4, 4]  # pipeline-fill chunk sizes (within 8-tile blocks)

F32 = mybir.dt.float32
BF16 = mybir.dt.bfloat16
AF = mybir.ActivationFunctionType
ALU = mybir.AluOpType
AX = mybir.AxisListType

_prog_cache = {}


def _build_program(n_tiles: int, W: int, C: int, tiers: tuple):
    # tiers: ((nres, c0, c1), ...) over relabeled (capacity-sorted) classes;
    # classes in [c0, c1) each own `nres` residues, contiguous in the residue
    # axis. W = 4 * R where R = total residues.
    R = W // 4
    # graduated chunk sizes: short chunks first so the DMA->exp->fold
    # pipeline fills fast, full CH-tile chunks in steady state
    n_chunks = n_tiles // CH
    assert n_tiles % CH == 0
    sizes = GRAD_SIZES + [CH] * ((n_tiles - sum(GRAD_SIZES)) // CH)
    assert sum(sizes) == n_tiles
    counts = {t: sizes.count(t) for t in sorted(set(sizes))}
    sched = []
    seen = {}
    for t in sizes:
        sched.append((t, seen.get(t, 0)))
        seen[t] = seen.get(t, 0) + 1
    nc = bacc.Bacc()

    lg_ds = {
        t: nc.dram_tensor(f"logits_{t}", [n, P, t * W], BF16, kind="ExternalInput")
        for t, n in counts.items()
    }
    oh_d = nc.dram_tensor("onehot", [P, n_tiles * C], BF16, kind="ExternalInput")
    out_d = nc.dram_tensor("out", [P, 1], F32, kind="ExternalOutput")

    with tile.TileContext(nc) as tc:
        with (
            tc.tile_pool(name="const", bufs=1) as cpool,
            tc.tile_pool(name="work", bufs=2) as wpool,
        ):
            oh_all = cpool.tile([P, n_tiles * C], BF16)
            em_all = cpool.tile([P, n_tiles * C], BF16)
            s_all = cpool.tile([P, n_tiles * C], BF16)
            sumem = cpool.tile([P, n_tiles], F32)
            z = cpool.tile([P, n_tiles], F32)
            num = cpool.tile([P, n_tiles], F32)
            den = cpool.tile([P, n_tiles], F32)
            g = cpool.tile([P, n_tiles * C // 4], BF16)
            g2 = cpool.tile([P, n_tiles * C // 4], BF16)

            def seg_tree(eng, src, out_view, off, m, nres, t, op, tag):
                # src[:, off*t:(off+m*nres)*t] viewed [p, m, g, t]; reduce
                # over g via pairwise folds into out_view [p, m, 1, t]. Odd
                # widths fold the last slot into slot 0 in place first.
                cur = src[:, off * t : (off + m * nres) * t].rearrange(
                    "p (m g t) -> p m g t", g=nres, t=t
                )
                width = nres
                si = 0
                while width > 1:
                    if width % 2 == 1:
                        eng.tensor_tensor(
                            cur[:, :, 0:1, :], cur[:, :, 0:1, :],
                            cur[:, :, width - 1 : width, :], op,
                        )
                        width -= 1
                    k = width // 2
                    if k == 1:
                        eng.tensor_tensor(
                            out_view, cur[:, :, 0:1, :], cur[:, :, 1:2, :], op
                        )
                    else:
                        dst = wpool.tile(
                            [P, m * k * TB], BF16, tag=f"{tag}h{si}",
                            name=f"{tag}h{si}", bufs=2,
                        )[:, 0 : m * k * t].rearrange("p (m g t) -> p m g t", g=k, t=t)
                        eng.tensor_tensor(
                            dst, cur[:, :, 0:k, :], cur[:, :, k : 2 * k, :], op
                        )
                        cur = dst
                    width = k
                    si += 1

            NQ = 4  # epilogue batches

            def cfold(buf_slice, out_f32, tag):
                # reduce over c of one TB-tile block [p, (c t)] via bf16
                # pairwise add-folds (TT is charged per operand: ~0.52/elem
                # vs 1.04 for TensorReduce)
                cur = buf_slice.rearrange("p (c t) -> p c t", t=TB)
                width = C
                si = 0
                while width > 2:
                    k = width // 2
                    nxt = wpool.tile(
                        [P, k * TB], BF16, tag=f"cf{si}", name=f"cf{si}",
                        bufs=2,
                    ).rearrange("p (c t) -> p c t", t=TB)
                    nc.vector.tensor_add(nxt, cur[:, 0:k, :], cur[:, k : 2 * k, :])
                    cur = nxt
                    width = k
                    si += 1
                nc.vector.tensor_add(
                    out_f32.rearrange("p (g t) -> p g t", g=1),
                    cur[:, 0:1, :], cur[:, 1:2, :],
                )

            def epilogue_q(h):
                # one quarter of the tokens: tiles [h*ntq, (h+1)*ntq)
                ntq = n_tiles // NQ
                lo, hi = h * ntq * C, (h + 1) * ntq * C
                emh, sh, ohh = em_all[:, lo:hi], s_all[:, lo:hi], oh_all[:, lo:hi]
                tl = slice(h * ntq, (h + 1) * ntq)
                cfold(emh, sumem[:, tl], "se")
                cfold(sh, z[:, tl], "zf")
                nc.vector.tensor_mul(g[:, :], emh, sh)
                nc.vector.tensor_mul(g2[:, :], g[:, :], ohh)
                cfold(g2[:, :], num[:, tl], "nf")
                nc.vector.tensor_mul(den[:, tl], sumem[:, tl], z[:, tl])

            tile_pos = 0
            epi_done = 0
            for si_, (t, ci) in enumerate(sched):
                lgf = wpool.tile([P, CH * W], BF16, tag="lg", name="lg", bufs=3)
                lg = lgf[:, 0 : t * W]
                nc.sync.dma_start(lg, lg_ds[t][ci])
                ef = wpool.tile([P, CH * W], BF16, tag="e", name="e", bufs=2)
                e = ef[:, 0 : t * W]
                # split the first t=4 ramp chunk's exp into two t=2 slices so
                # DVE folds can start on the first half sooner
                esplit = 2 if (si_ <= 4 and t == 4) else t
                for ss in range(0, t, esplit):
                    nc.scalar.activation(
                        e.rearrange("p (w t) -> p w t", t=t)[:, :, ss : ss + esplit],
                        lg.rearrange("p (w t) -> p w t", t=t)[:, :, ss : ss + esplit],
                        AF.Exp,
                    )
                half = W // 2 * t
                quart = R * t

                with nc.allow_low_precision("bf16 folds; noise averages out"):
                    # fold2 writes t-slices of BLOCK-wide residue buffers so
                    # the tier trees run once per TB-tile block regardless of
                    # the (graduated) chunk size
                    bb0, tt0 = tile_pos // TB, tile_pos % TB
                    if tt0 == 0:
                        fm2f = wpool.tile([P, R * TB], BF16, tag="fm2", name="fm2", bufs=2)
                        fs2f = wpool.tile([P, R * TB], BF16, tag="fs2", name="fs2", bufs=2)
                        blkbufs = (fm2f, fs2f)
                    else:
                        fm2f, fs2f = blkbufs
                    fm2o = fm2f.rearrange("p (r t) -> p r t", t=TB)[:, :, tt0 : tt0 + t]
                    fs2o = fs2f.rearrange("p (r t) -> p r t", t=TB)[:, :, tt0 : tt0 + t]
                    fm1f = wpool.tile([P, W // 2 * CH], BF16, tag="fm1", name="fm1", bufs=2)
                    fm1 = fm1f[:, 0:half]
                    fs1f = wpool.tile([P, W // 2 * CH], BF16, tag="fs1", name="fs1", bufs=2)
                    fs1 = fs1f[:, 0:half]
                    e3 = e.rearrange("p (w t) -> p w t", t=t)
                    fm13 = fm1.rearrange("p (w t) -> p w t", t=t)
                    fs13 = fs1.rearrange("p (w t) -> p w t", t=t)
                    for ss in range(0, t, esplit):
                        sl = slice(ss, ss + esplit)
                        nc.vector.tensor_max(
                            fm13[:, :, sl], e3[:, 0 : W // 2, sl], e3[:, W // 2 :, sl]
                        )
                        nc.vector.tensor_max(
                            fm2o[:, :, sl] if esplit != t else fm2o,
                            fm13[:, 0:R, sl], fm13[:, R : 2 * R, sl],
                        )
                        nc.vector.tensor_add(
                            fs13[:, :, sl], e3[:, 0 : W // 2, sl], e3[:, W // 2 :, sl]
                        )
                        nc.vector.tensor_add(
                            fs2o[:, :, sl] if esplit != t else fs2o,
                            fs13[:, 0:R, sl], fs13[:, R : 2 * R, sl],
                        )

                    # ragged tier reduce over g = nres residues: pairwise TT
                    # trees on DVE, once per completed TB-tile block
                    if (tile_pos + t) % TB == 0:
                        bb = tile_pos // TB
                        em = em_all[:, bb * C * TB : (bb + 1) * C * TB]
                        sc = s_all[:, bb * C * TB : (bb + 1) * C * TB]
                        for eng, src, dstbuf, op, nm in (
                            (nc.vector, fm2f, em, ALU.max, "m"),
                            (nc.vector, fs2f, sc, ALU.add, "s"),
                        ):
                            off = 0
                            for nres, c0, c1 in tiers:
                                m = c1 - c0
                                ov = dstbuf[:, c0 * TB : c1 * TB].rearrange(
                                    "p (m g t) -> p m g t", g=1, t=TB
                                )
                                seg_tree(
                                    eng, src, ov, off, m, nres, TB, op, f"t{nm}{nres}"
                                )
                                off += m * nres

                tile_pos += t
                nq4 = n_tiles * C // 4
                if si_ == 3:
                    nc.sync.dma_start(oh_all[:, 0 : 2 * nq4], oh_d[:, 0 : 2 * nq4])
                if si_ == 7:
                    nc.sync.dma_start(oh_all[:, 2 * nq4 :], oh_d[:, 2 * nq4 :])
                # emit epilogue quarters as soon as their tiles are reduced
                # (same-engine in-order: no cross-engine wait remains)
                ntq = n_tiles // NQ
                while epi_done < NQ - 1 and tile_pos >= (epi_done + 1) * ntq:
                    with nc.allow_low_precision("bf16 epilogue"):
                        epilogue_q(epi_done)
                    epi_done += 1

            with nc.allow_low_precision("bf16 epilogue"):
                while epi_done < NQ:
                    epilogue_q(epi_done)
                    epi_done += 1
                lnum = cpool.tile([P, n_tiles], F32)
                nc.scalar.activation(lnum[:, :], num[:, :], AF.Ln)
                lden = cpool.tile([P, n_tiles], F32)
                nc.scalar.activation(lden[:, :], den[:, :], AF.Ln)
                term = cpool.tile([P, n_tiles], F32)
                nc.vector.tensor_sub(term[:, :], lnum[:, :], lden[:, :])
                acc = cpool.tile([P, 1], F32)
                nc.vector.tensor_reduce(acc[:, :], term[:, :], axis=AX.X, op=ALU.add)
                nc.sync.dma_start(out_d[:, :], acc[:, :])

    nc.finalize()
    return nc


def _prepare(logits, labels, mask_matrix):
    B, S, F = logits.shape
    C = mask_matrix.shape[1]
    n_tok = B * S
    tok_per_core = n_tok // N_CORES
    n_tiles = tok_per_core // P
    n_chunks = n_tiles // CH

    seg = np.asarray(mask_matrix).argmax(axis=1)
    members0 = [np.nonzero(seg == c)[0] for c in range(C)]
    sizes = np.array([len(m) for m in members0])
    caps = np.maximum(4, -(-sizes // 4) * 4)  # 2-level fold-stable capacity
    perm_c = np.argsort(caps, kind="stable")  # relabel by ascending capacity
    members = [members0[c] for c in perm_c]
    caps = caps[perm_c].astype(np.int64)

    R = int(caps.sum()) // 4
    W = 4 * R
    nres = caps // 4
    offs = np.concatenate([[0], np.cumsum(nres)])  # residue offset per class

    tier_list = []
    c0 = 0
    for c in range(1, C + 1):
        if c == C or nres[c] != nres[c0]:
            tier_list.append((int(nres[c0]), c0, c))
            c0 = c
    tiers = tuple(tier_list)

    # slot table: member j of class c -> residue off_c + j % nres_c,
    # level-slot j // nres_c in {0,1,2,3}; slot = residue + level * R
    perm = np.full(W, -1, dtype=np.int64)
    for c, mem in enumerate(members):
        n = int(nres[c])
        j = np.arange(len(mem))
        perm[(offs[c] + j % n) + (j // n) * R] = mem

    # permuted bf16 logits, [core, chunk, p, w, t] (w-major, tile-minor),
    # grouped by graduated chunk size
    sizes = GRAD_SIZES + [CH] * ((n_tiles - sum(GRAD_SIZES)) // CH)
    lg2 = np.asarray(logits, dtype=np.float32).reshape(n_tok, F)
    lgp = np.full((n_tok, W), -1000.0, dtype=np.float32)
    valid = perm >= 0
    lgp[:, valid] = lg2[:, perm[valid]]
    lgp = lgp.astype(ml_dtypes.bfloat16)
    lgp = lgp.reshape(N_CORES, n_tiles, P, W)
    lgs = {t: [] for t in sorted(set(sizes))}
    pos = 0
    for t in sizes:
        blk = lgp[:, pos : pos + t].transpose(0, 2, 3, 1)  # [k, P, W, t]
        lgs[t].append(np.ascontiguousarray(blk).reshape(N_CORES, P, t * W))
        pos += t
    lgs = {t: np.stack(v, axis=1) for t, v in lgs.items()}  # [k, n_t, P, t*W]

    # one-hot labels, [core, p, chunk, c, t] (matches em/s chunk layout)
    inv_perm = np.empty(C, dtype=np.int64)
    inv_perm[perm_c] = np.arange(C)
    lab = inv_perm[np.asarray(labels).reshape(-1).astype(np.int64)]
    n_blk = n_tiles // TB
    lab = lab.reshape(N_CORES, n_blk, TB, P)
    oh = np.zeros((N_CORES, n_blk, C, TB, P), dtype=ml_dtypes.bfloat16)
    k, ch, t, p = np.meshgrid(
        np.arange(N_CORES), np.arange(n_blk), np.arange(TB), np.arange(P),
        indexing="ij",
    )
    oh[k, ch, lab, t, p] = 1.0
    oh = np.ascontiguousarray(oh.transpose(0, 4, 1, 2, 3)).reshape(
        N_CORES, P, n_tiles * C
    )

    return lgs, oh, tiers, n_tiles, W, C, n_tok


def _run(logits, labels, mask_matrix, **spmd_kwargs):
    lgs, oh, tiers, n_tiles, W, C, n_tok = _prepare(logits, labels, mask_matrix)
    key = (n_tiles, W, C, tiers)
    if key not in _prog_cache:
        _prog_cache[key] = _build_program(*key)
    nc = _prog_cache[key]
    in_maps = [
        {**{f"logits_{t}": a[k] for t, a in lgs.items()}, "onehot": oh[k]}
        for k in range(N_CORES)
    ]
    res = run_bass_kernel_spmd(nc, in_maps, core_ids=list(range(N_CORES)), **spmd_kwargs)
    total = np.float64(0.0)
    for r in res.results:
        total += np.float64(r["out"].sum(dtype=np.float64))
    loss = np.float32(-0.5 * total / n_tok)
    return loss, res


def kernel(logits, labels, mask_matrix):
    loss, _ = _run(logits, labels, mask_matrix)
    return loss


# revision 34
# speedup vs baseline: 1.0061x; 1.0061x over previous
"""MixLoss Trainium2 kernel.

loss = 0.5*(ce + nll) over tokens, with
  ce  = -mean[ log_softmax_c(segment_max_f(logits))[label] ]
  nll = -mean[ log((softmax_f(logits) @ mask)[label]) ]

Data-parallel over 8 cores (batch split). Per core: 8192 tokens = 64 tiles
of 128 (tokens on SBUF partitions).

The host permutes the fine-class axis into a 2-level FOLD-STABLE layout:
each coarse class gets a capacity cap_c = 4*ceil(size_c/4); class c owns
residues [off_c, off_c + cap_c/4) of R = W/4, and its members occupy slots
{r + k*R : k in 0..4} for its residues r (pads hold logit -1000 -> exp 0,
neutral for both the group max and the group sum since all real E > 0).
Within a chunk of CH tiles the SBUF layout is w-major/tile-minor
([p, w*CH + t]) so every fold is a single fully-packed bf16 TensorTensor
(DVE 2x mode: 0.52 ns/elem in the cost model vs 1.04 for TensorReduce):

  fold1:  [0:W/2*CH] (op) [W/2*CH:W*CH]      (pairs w, w+2R)
  fold2:  [0:W/4*CH] (op) [W/4*CH:W/2*CH]    (pairs w, w+R)
  ragged tier reduce over g = cap/4 residues: pairwise TT trees on the
  Pool engine (otherwise idle) -> em/s slices of [p, ch*C*CH + c*CH + t]

Tail epilogue (batched once, off the pipeline's critical path):
  sum_em = sum_c EM, Z = sum_c S (Z = full row-sum of E since the segments
  partition the row), G = EM*S*onehot, num = sum_c G = EM[lab]*S[lab],
  den = sum_em*Z, term = ln(num) - ln(den) = logp_max[lab]+logp_coarse[lab].
Host sums partials: loss = -0.5 * sum(term) / n_tok.

bf16 logits / E / EM / S storage gives ~0.4% zero-mean rounding noise per
token which averages out over 65536 tokens (baseline did the same).
"""

import ml_dtypes
import numpy as np

import concourse.bacc as bacc
import concourse.mybir as mybir
from concourse import tile
from concourse.bass_utils import run_bass_kernel_spmd

N_CORES = 8
P = 128  # SBUF partitions = tokens per tile
CH = 8   # tiles per chunk (steady state)
TB = 16  # tiles per tree/epilogue block
GRAD_SIZES = [2, 2, 4, 4, 4]  # pipeline-fill chunk sizes (within 8-tile blocks)

F32 = mybir.dt.float32
BF16 = mybir.dt.bfloat16
AF = mybir.ActivationFunctionType
ALU = mybir.AluOpType
AX = mybir.AxisListType

_prog_cache = {}


def _build_program(n_tiles: int, W: int, C: int, tiers: tuple):
    # tiers: ((nres, c0, c1), ...) over relabeled (capacity-sorted) classes;
    # classes in [c0, c1) each own `nres` residues, contiguous in the residue
    # axis. W = 4 * R where R = total residues.
    R = W // 4
    # graduated chunk sizes: short chunks first so the DMA->exp->fold
    # pipeline fills fast, full CH-tile chunks in steady state
    n_chunks = n_tiles // CH
    assert n_tiles % CH == 0
    sizes = GRAD_SIZES + [CH] * ((n_tiles - sum(GRAD_SIZES)) // CH)
    assert sum(sizes) == n_tiles
    counts = {t: sizes.count(t) for t in sorted(set(sizes))}
    sched = []
    seen = {}
    for t in sizes:
        sched.append((t, seen.get(t, 0)))
        seen[t] = seen.get(t, 0) + 1
    nc = bacc.Bacc()

    lg_ds = {
        t: nc.dram_tensor(f"logits_{t}", [n, P, t * W], BF16, kind="ExternalInput")
        for t, n in counts.items()
    }
    oh_d = nc.dram_tensor("onehot", [P, n_tiles * C], BF16, kind="ExternalInput")
    out_d = nc.dram_tensor("out", [P, 1], F32, kind="ExternalOutput")

    with tile.TileContext(nc) as tc:
        with (
            tc.tile_pool(name="const", bufs=1) as cpool,
            tc.tile_pool(name="work", bufs=2) as wpool,
        ):
            oh_all = cpool.tile([P, n_tiles * C], BF16)
            em_all = cpool.tile([P, n_tiles * C], BF16)
            s_all = cpool.tile([P, n_tiles * C], BF16)
            sumem = cpool.tile([P, n_tiles], F32)
            z = cpool.tile([P, n_tiles], F32)
            num = cpool.tile([P, n_tiles], F32)
            den = cpool.tile([P, n_tiles], F32)
            g = cpool.tile([P, n_tiles * C // 4], BF16)
            g2 = cpool.tile([P, n_tiles * C // 4], BF16)

            def seg_tree(eng, src, out_view, off, m, nres, t, op, tag):
                # src[:, off*t:(off+m*nres)*t] viewed [p, m, g, t]; reduce
                # over g via pairwise folds into out_view [p, m, 1, t]. Odd
                # widths fold the last slot into slot 0 in place first.
                cur = src[:, off * t : (off + m * nres) * t].rearrange(
                    "p (m g t) -> p m g t", g=nres, t=t
                )
                width = nres
                si = 0
                while width > 1:
                    if width % 2 == 1:
                        eng.tensor_tensor(
                            cur[:, :, 0:1, :], cur[:, :, 0:1, :],
                            cur[:, :, width - 1 : width, :], op,
                        )
                        width -= 1
                    k = width // 2
                    if k == 1:
                        eng.tensor_tensor(
                            out_view, cur[:, :, 0:1, :], cur[:, :, 1:2, :], op
                        )
                    else:
                        dst = wpool.tile(
                            [P, m * k * TB], BF16, tag=f"{tag}h{si}",
                            name=f"{tag}h{si}", bufs=2,
                        )[:, 0 : m * k * t].rearrange("p (m g t) -> p m g t", g=k, t=t)
                        eng.tensor_tensor(
                            dst, cur[:, :, 0:k, :], cur[:, :, k : 2 * k, :], op
                        )
                        cur = dst
                    width = k
                    si += 1

            NQ = 4  # epilogue batches

            def cfold(buf_slice, out_f32, tag):
                # reduce over c of one TB-tile block [p, (c t)] via bf16
                # pairwise add-folds (TT is charged per operand: ~0.52/elem
                # vs 1.04 for TensorReduce)
                cur = buf_slice.rearrange("p (c t) -> p c t", t=TB)
                width = C
                si = 0
                while width > 2:
                    k = width // 2
                    nxt = wpool.tile(
                        [P, k * TB], BF16, tag=f"cf{si}", name=f"cf{si}",
                        bufs=2,
                    ).rearrange("p (c t) -> p c t", t=TB)
                    nc.vector.tensor_add(nxt, cur[:, 0:k, :], cur[:, k : 2 * k, :])
                    cur = nxt
                    width = k
                    si += 1
                nc.vector.tensor_add(
                    out_f32.rearrange("p (g t) -> p g t", g=1),
                    cur[:, 0:1, :], cur[:, 1:2, :],
                )

            def epilogue_q(h):
                # one quarter of the tokens: tiles [h*ntq, (h+1)*ntq)
                ntq = n_tiles // NQ
                lo, hi = h * ntq * C, (h + 1) * ntq * C
                emh, sh, ohh = em_all[:, lo:hi], s_all[:, lo:hi], oh_all[:, lo:hi]
                tl = slice(h * ntq, (h + 1) * ntq)
                cfold(emh, sumem[:, tl], "se")
                cfold(sh, z[:, tl], "zf")
                nc.vector.tensor_mul(g[:, :], emh, sh)
                nc.vector.tensor_mul(g2[:, :], g[:, :], ohh)
                cfold(g2[:, :], num[:, tl], "nf")
                nc.vector.tensor_mul(den[:, tl], sumem[:, tl], z[:, tl])

            tile_pos = 0
            epi_done = 0
            for si_, (t, ci) in enumerate(sched):
                lgf = wpool.tile([P, CH * W], BF16, tag="lg", name="lg", bufs=3)
                lg = lgf[:, 0 : t * W]
                nc.sync.dma_start(lg, lg_ds[t][ci])
                ef = wpool.tile([P, CH * W], BF16, tag="e", name="e", bufs=2)
                e = ef[:, 0 : t * W]
                # split the first t=4 ramp chunk's exp into two t=2 slices so
                # DVE folds can start on the first half sooner
                esplit = 2 if (si_ <= 4 and t == 4) else t
                for ss in range(0, t, esplit):
                    nc.scalar.activation(
                        e.rearrange("p (w t) -> p w t", t=t)[:, :, ss : ss + esplit],
                        lg.rearrange("p (w t) -> p w t", t=t)[:, :, ss : ss + esplit],
                        AF.Exp,
                    )
                half = W // 2 * t
                quart = R * t

                with nc.allow_low_precision("bf16 folds; noise averages out"):
                    # fold2 writes t-slices of BLOCK-wide residue buffers so
                    # the tier trees run once per TB-tile block regardless of
                    # the (graduated) chunk size
                    bb0, tt0 = tile_pos // TB, tile_pos % TB
                    if tt0 == 0:
                        fm2f = wpool.tile([P, R * TB], BF16, tag="fm2", name="fm2", bufs=2)
                        fs2f = wpool.tile([P, R * TB], BF16, tag="fs2", name="fs2", bufs=2)
                        blkbufs = (fm2f, fs2f)
                    else:
                        fm2f, fs2f = blkbufs
                    fm2o = fm2f.rearrange("p (r t) -> p r t", t=TB)[:, :, tt0 : tt0 + t]
                    fs2o = fs2f.rearrange("p (r t) -> p r t", t=TB)[:, :, tt0 : tt0 + t]
                    fm1f = wpool.tile([P, W // 2 * CH], BF16, tag="fm1", name="fm1", bufs=2)
                    fm1 = fm1f[:, 0:half]
                    fs1f = wpool.tile([P, W // 2 * CH], BF16, tag="fs1", name="fs1", bufs=2)
                    fs1 = fs1f[:, 0:half]
                    e3 = e.rearrange("p (w t) -> p w t", t=t)
                    fm13 = fm1.rearrange("p (w t) -> p w t", t=t)
                    fs13 = fs1.rearrange("p (w t) -> p w t", t=t)
                    for ss in range(0, t, esplit):
                        sl = slice(ss, ss + esplit)
                        nc.vector.tensor_max(
                            fm13[:, :, sl], e3[:, 0 : W // 2, sl], e3[:, W // 2 :, sl]
                        )
                        nc.vector.tensor_max(
                            fm2o[:, :, sl] if esplit != t else fm2o,
                            fm13[:, 0:R, sl], fm13[:, R : 2 * R, sl],
                        )
                        nc.vector.tensor_add(
                            fs13[:, :, sl], e3[:, 0 : W // 2, sl], e3[:, W // 2 :, sl]
                        )
                        nc.vector.tensor_add(
                            fs2o[:, :, sl] if esplit != t else fs2o,
                            fs13[:, 0:R, sl], fs13[:, R : 2 * R, sl],
                        )

                    # ragged tier reduce over g = nres residues: pairwise TT
                    # trees on DVE, once per completed TB-tile block
                    if (tile_pos + t) % TB == 0:
                        bb = tile_pos // TB
                        em = em_all[:, bb * C * TB : (bb + 1) * C * TB]
                        sc = s_all[:, bb * C * TB : (bb + 1) * C * TB]
                        for eng, src, dstbuf, op, nm in (
                            (nc.vector, fm2f, em, ALU.max, "m"),
                            (nc.vector, fs2f, sc, ALU.add, "s"),
                        ):
                            off = 0
                            for nres, c0, c1 in tiers:
                                m = c1 - c0
                                ov = dstbuf[:, c0 * TB : c1 * TB].rearrange(
                                    "p (m g t) -> p m g t", g=1, t=TB
                                )
                                seg_tree(
                                    eng, src, ov, off, m, nres, TB, op, f"t{nm}{nres}"
                                )
                                off += m * nres

                tile_pos += t
                nq4 = n_tiles * C // 4
                for qi, at in enumerate((2, 5, 7, 9)):
                    if si_ == at:
                        nc.sync.dma_start(
                            oh_all[:, qi * nq4 : (qi + 1) * nq4],
                            oh_d[:, qi * nq4 : (qi + 1) * nq4],
                        )
                # emit epilogue quarters as soon as their tiles are reduced
                # (same-engine in-order: no cross-engine wait remains)
                ntq = n_tiles // NQ
                while epi_done < NQ - 1 and tile_pos >= (epi_done + 1) * ntq:
                    with nc.allow_low_precision("bf16 epilogue"):
                        epilogue_q(epi_done)
                    epi_done += 1

            with nc.allow_low_precision("bf16 epilogue"):
                while epi_done < NQ:
                    epilogue_q(epi_done)
                    epi_done += 1
                lnum = cpool.tile([P, n_tiles], F32)
                nc.scalar.activation(lnum[:, :], num[:, :], AF.Ln)
                lden = cpool.tile([P, n_tiles], F32)
                nc.scalar.activation(lden[:, :], den[:, :], AF.Ln)
                term = cpool.tile([P, n_tiles], F32)
                nc.vector.tensor_sub(term[:, :], lnum[:, :], lden[:, :])
                acc = cpool.tile([P, 1], F32)
                nc.vector.tensor_reduce(acc[:, :], term[:, :], axis=AX.X, op=ALU.add)
                nc.sync.dma_start(out_d[:, :], acc[:, :])

    nc.finalize()
    return nc


def _prepare(logits, labels, mask_matrix):
    B, S, F = logits.shape
    C = mask_matrix.shape[1]
    n_tok = B * S
    tok_per_core = n_tok // N_CORES
    n_tiles = tok_per_core // P
    n_chunks = n_tiles // CH

    seg = np.asarray(mask_matrix).argmax(axis=1)
    members0 = [np.nonzero(seg == c)[0] for c in range(C)]
    sizes = np.array([len(m) for m in members0])
    caps = np.maximum(4, -(-sizes // 4) * 4)  # 2-level fold-stable capacity
    perm_c = np.argsort(caps, kind="stable")  # relabel by ascending capacity
    members = [members0[c] for c in perm_c]
    caps = caps[perm_c].astype(np.int64)

    R = int(caps.sum()) // 4
    W = 4 * R
    nres = caps // 4
    offs = np.concatenate([[0], np.cumsum(nres)])  # residue offset per class

    tier_list = []
    c0 = 0
    for c in range(1, C + 1):
        if c == C or nres[c] != nres[c0]:
            tier_list.append((int(nres[c0]), c0, c))
            c0 = c
    tiers = tuple(tier_list)

    # slot table: member j of class c -> residue off_c + j % nres_c,
    # level-slot j // nres_c in {0,1,2,3}; slot = residue + level * R
    perm = np.full(W, -1, dtype=np.int64)
    for c, mem in enumerate(members):
        n = int(nres[c])
        j = np.arange(len(mem))
        perm[(offs[c] + j % n) + (j // n) * R] = mem

    # permuted bf16 logits, [core, chunk, p, w, t] (w-major, tile-minor),
    # grouped by graduated chunk size
    sizes = GRAD_SIZES + [CH] * ((n_tiles - sum(GRAD_SIZES)) // CH)
    lg2 = np.asarray(logits, dtype=np.float32).reshape(n_tok, F)
    lgp = np.full((n_tok, W), -1000.0, dtype=np.float32)
    valid = perm >= 0
    lgp[:, valid] = lg2[:, perm[valid]]
    lgp = lgp.astype(ml_dtypes.bfloat16)
    lgp = lgp.reshape(N_CORES, n_tiles, P, W)
    lgs = {t: [] for t in sorted(set(sizes))}
    pos = 0
    for t in sizes:
        blk = lgp[:, pos : pos + t].transpose(0, 2, 3, 1)  # [k, P, W, t]
        lgs[t].append(np.ascontiguousarray(blk).reshape(N_CORES, P, t * W))
        pos += t
    lgs = {t: np.stack(v, axis=1) for t, v in lgs.items()}  # [k, n_t, P, t*W]

    # one-hot labels, [core, p, chunk, c, t] (matches em/s chunk layout)
    inv_perm = np.empty(C, dtype=np.int64)
    inv_perm[perm_c] = np.arange(C)
    lab = inv_perm[np.asarray(labels).reshape(-1).astype(np.int64)]
    n_blk = n_tiles // TB
    lab = lab.reshape(N_CORES, n_blk, TB, P)
    oh = np.zeros((N_CORES, n_blk, C, TB, P), dtype=ml_dtypes.bfloat16)
    k, ch, t, p = np.meshgrid(
        np.arange(N_CORES), np.arange(n_blk), np.arange(TB), np.arange(P),
        indexing="ij",
    )
    oh[k, ch, lab, t, p] = 1.0
    oh = np.ascontiguousarray(oh.transpose(0, 4, 1, 2, 3)).reshape(
        N_CORES, P, n_tiles * C
    )

    return lgs, oh, tiers, n_tiles, W, C, n_tok


def _run(logits, labels, mask_matrix, **spmd_kwargs):
    lgs, oh, tiers, n_tiles, W, C, n_tok = _prepare(logits, labels, mask_matrix)
    key = (n_tiles, W, C, tiers)
    if key not in _prog_cache:
        _prog_cache[key] = _build_program(*key)
    nc = _prog_cache[key]
    in_maps = [
        {**{f"logits_{t}": a[k] for t, a in lgs.items()}, "onehot": oh[k]}
        for k in range(N_CORES)
    ]
    res = run_bass_kernel_spmd(nc, in_maps, core_ids=list(range(N_CORES)), **spmd_kwargs)
    total = np.float64(0.0)
    for r in res.results:
        total += np.float64(r["out"].sum(dtype=np.float64))
    loss = np.float32(-0.5 * total / n_tok)
    return loss, res


def kernel(logits, labels, mask_matrix):
    loss, _ = _run(logits, labels, mask_matrix)
    return loss
